# revision 11
# baseline (speedup 1.0000x reference)
"""BitLinear (ternary absmean-quantized linear) on 8 TRN2 NeuronCores.

Reference math (fp32):
    gamma = mean(|W|)
    Wq    = round(clip(W / (gamma + 1e-5), -1, 1))   # ternary {-1, 0, 1}
    out   = einsum('bsi,oi->bso', x, Wq)             # x @ Wq.T

Strategy (v2, fp8 DoubleRow):
  2D sharding: tokens split T=2 ways x features F=4 ways (core c -> token
  group c//4, feature shard c%4). Per core: 4096 tokens x 1024 features,
  K=4096 contraction.

  The matmul runs in fp8e4 with MatmulPerfMode.DoubleRow (2 fp8 weights per
  PE cell -> 2 MACs/cell/cycle, ~2x bf16 ALU rate). Ternary Wq is exact in
  fp8e4. x is quantized host-side to e4m3 "hi" slabs; e4m3 alone is too
  lossy (fro rel err 0.024 > 2e-2 gate), so C=12 of the 32 k-slabs also get
  an e4m3 "lo" residual slab (hi+lo is near-exact), bringing the error to
  ~0.019 at 1.375x the matmul work of a plain fp8 pass (= 0.69x bf16 work).

  A DoubleRow matmul contracts TWO k-slabs: lhsT = Wq[:, (a,a+1), ft*128:+128]
  (stationary, reused across 8 moving matmuls to amortize LDWEIGHTS), rhs =
  x[:, (sa,sa+1), tc*256:+256]. Correction pairs reuse the same consecutive
  Wq slab pair against (lo_a, lo_a+1) x-slabs, so resident Wq stays a plain
  [128, 32, N_CORE] fp8 tile with no duplication.

  Quantization of W happens on device: gamma partials are abs-sums of a
  512-column slice per core (the 8 slices tile the full W), combined by a
  tiny [128,1] AllReduce; threshold t = (gamma+1e-5)/2; Wq = (w>t) - (w<-t)
  on DVE, written as fp8 into the resident tile while the stream runs.

  PSUM: each [128, 512] f32 tile = one 2KB bank = one accumulation group
  covering 2 token-chunks (start=True clears the bank's has_written bits;
  per-element first-write-overwrite makes the second chunk's first matmul
  safe with start=False). 4 banks per ft sweep -> 2 fts in flight.

  Output is drained as bf16 [features, tokens] (transposed); the host
  upcasts/transposes and assembles the full [4,2048,4096] f32 output.
"""

import numpy as np
import ml_dtypes

NCORES = 8

# Full-problem dims (hardcoded per the harness contract).
B, S, D_IN, D_OUT = 4, 2048, 4096, 4096
M_TOTAL = B * S              # 8192 tokens

T_SHARD = 2                  # token split
F_SHARD = 4                  # feature split
M_CORE = M_TOTAL // T_SHARD  # 4096 tokens per core
N_CORE = D_OUT // F_SHARD    # 1024 features per core
KT = D_IN // 128             # 32 k-slabs
CORR_START = 16              # k-slabs [CORR_START, KT) get lo-residual slabs
NSLAB = KT + (KT - CORR_START)   # 48 x-slabs (32 hi + 16 lo)
MG = 2                       # m-groups per core
M_GRP = M_CORE // MG         # 2048 tokens per m-group
TCH = 256                    # tokens per matmul (moving free = 2*256)
NGAMMA = D_IN // NCORES      # 512-column gamma slice per core

_COMPILED = None
LAST_RESULTS = None


def _pairs(kt=KT, corr_start=CORR_START):
    """DoubleRow pair schedule: (wq slab start, x slab start)."""
    ps = [(2 * p, 2 * p) for p in range(kt // 2)]
    ncorr = kt - corr_start
    ps += [(corr_start + 2 * j, kt + 2 * j) for j in range(ncorr // 2)]
    return ps


def build_module(m_grp=M_GRP, mg=MG, n_core=N_CORE, kt=KT,
                 corr_start=CORR_START, ncores=NCORES, repeat=1,
                 loop_reps=1, use_collective=True):
    """loop_reps>1 wraps the `repeat` unrolled reps in a tc.For_i hardware
    loop (loop_reps iterations) — used for steady-state timing with a large
    execution count at constant instruction count."""
    import concourse.bass as bass  # noqa: F401
    import concourse.mybir as mybir
    import concourse.tile as tile
    from concourse import bacc
    from concourse import bass_isa

    f32 = mybir.dt.float32
    bf16 = mybir.dt.bfloat16
    fp8 = mybir.dt.float8e4

    k = kt * 128
    nslab = kt + (kt - corr_start)
    pairs = _pairs(kt, corr_start)
    npair = len(pairs)
    nft = n_core // 128
    ntch = m_grp // TCH          # token chunks per m-group
    nbank = ntch // 2            # psum banks per ft sweep
    m_core = m_grp * mg
    ng = k // ncores             # gamma slice width
    G_CHUNK = 2
    G_CHUNKS = kt // G_CHUNK
    # gamma normalizer: the ncores gamma slices tile the full weight matrix
    N_ELEMS = float(k * ng * ncores)

    nc = bacc.Bacc("TRN2", target_bir_lowering=False, debug=False,
                   num_devices=ncores)
    xS = nc.dram_tensor("xS", [nslab * 128, m_core], fp8, kind="ExternalInput")
    WT = nc.dram_tensor("WT", [k, n_core], f32, kind="ExternalInput")
    Wg = nc.dram_tensor("Wg", [k, ng], f32, kind="ExternalInput")
    outT = nc.dram_tensor("outT", [n_core, m_core], bf16, kind="ExternalOutput")

    ts = bass.ts

    with tile.TileContext(nc) as tc:
        with (
            tc.tile_pool(name="xpool", bufs=min(npair + 1, 2 * npair)) as xpool,
            tc.tile_pool(name="gpool", bufs=2) as gpool,
            tc.tile_pool(name="wqpool", bufs=2) as wqpool,
            tc.tile_pool(name="wpool", bufs=3) as wpool,
            tc.tile_pool(name="spool", bufs=4) as spool,
            tc.tile_pool(name="opool", bufs=6) as opool,
            tc.tile_pool(name="small", bufs=2) as small,
            tc.tile_pool(name="pmain", bufs=8, space="PSUM") as pmain,
            tc.tile_pool(name="dram", bufs=2, space="DRAM") as dram,
        ):
          with tc.tile_pool(name="cpool", bufs=1) as cpool:
            bias_p = cpool.tile([128, 1], f32, name="bias_p")
            nc.gpsimd.memset(bias_p[:], 0.5e-5)
            bias_n = cpool.tile([128, 1], f32, name="bias_n")
            nc.gpsimd.memset(bias_n[:], -0.5e-5)
          with __import__("contextlib").ExitStack() as lstack:
           if loop_reps > 1:
               lstack.enter_context(tc.For_i(0, loop_reps))
           for _rep in range(repeat):
            # ---- gamma: local abs-sum over this core's 512-col slice ----
            # ACT queue: idle during the main loop, so in steady state rep
            # i+1's gamma chain overlaps rep i's matmuls.
            acc = small.tile([128, G_CHUNKS], f32)
            for j in range(G_CHUNKS):
                gsl = gpool.tile([128, G_CHUNK, ng], f32, tag="gsl")
                src = Wg[j * G_CHUNK * 128:(j + 1) * G_CHUNK * 128, :]
                nc.scalar.dma_start(
                    gsl[:], src.rearrange("(t p) c -> p t c", p=128))
                gscr = gpool.tile([128, G_CHUNK, ng], bf16, tag="gscr")
                nc.scalar.activation(
                    gscr[:], gsl[:], mybir.ActivationFunctionType.Abs,
                    accum_out=acc[:, j:j + 1])
            gpart = small.tile([128, 1], f32)
            gscr2 = small.tile([128, G_CHUNKS], bf16)
            nc.scalar.activation(
                gscr2[:], acc[:], mybir.ActivationFunctionType.Abs,
                accum_out=gpart[:])

            # ---- tiny AllReduce of per-partition partials ----
            gsum = small.tile([128, 1], f32)
            if ncores > 1 and use_collective:
                cin = dram.tile([128, 1], f32)
                nc.scalar.dma_start(cin[:], gpart[:])
                cout = dram.tile([128, 1], f32, tag="cout", name=f"cout{_rep}")
                nc.gpsimd.collective_compute(
                    "AllReduce", mybir.AluOpType.add,
                    replica_groups=[list(range(ncores))],
                    ins=[cin[:].opt()], outs=[cout[:].opt()])
                nc.scalar.dma_start(gsum[:], cout[:])
            else:
                nc.scalar.copy(gsum[:], gpart[:])

            gtot = small.tile([128, 1], f32)
            nc.gpsimd.partition_all_reduce(
                gtot[:], gsum[:], channels=128, reduce_op=bass_isa.ReduceOp.add)

            # threshold t = 0.5 * (gamma + 1e-5); Wq = (w > t) - (w < -t)
            tsb = small.tile([128, 1], f32)
            nc.scalar.activation(
                tsb[:], gtot[:], mybir.ActivationFunctionType.Identity,
                bias=bias_p[:], scale=0.5 / N_ELEMS)
            ntsb = small.tile([128, 1], f32)
            nc.scalar.activation(
                ntsb[:], gtot[:], mybir.ActivationFunctionType.Identity,
                bias=bias_n[:], scale=-0.5 / N_ELEMS)

            # ---- W stream -> resident fp8 ternary WqS [128, kt, n_core] ----
            # x pair tiles for m-group 0 are interleaved into the same sync
            # queue in pair order so the ft0 sweep can start on pair 0 as
            # soon as its two slabs are quantized.
            wqs = wqpool.tile([128, kt, n_core], fp8)
            xt = {}
            qdone = set()
            for pi, (wa, xa) in enumerate(pairs):
                xt[(0, pi)] = xpool.tile([128, 2, m_grp], fp8, tag="xt",
                                         name=f"xt0_{pi}")
                nc.sync.dma_start(
                    xt[(0, pi)][:],
                    xS[xa * 128:(xa + 2) * 128, 0:m_grp]
                    .rearrange("(t p) m -> p t m", p=128))
                for sl in (wa, wa + 1):
                    if sl in qdone:
                        continue
                    qdone.add(sl)
                    wtmp = wpool.tile([128, n_core], f32, tag="wtmp")
                    nc.sync.dma_start(wtmp[:], WT[ts(sl, 128), :])
                    neg = spool.tile([128, n_core], fp8, tag="neg")
                    nc.vector.tensor_scalar(
                        neg[:], wtmp[:], ntsb[:], None,
                        mybir.AluOpType.is_lt)
                    nc.vector.scalar_tensor_tensor(
                        wqs[:, sl, :], wtmp[:], tsb[:], neg[:],
                        mybir.AluOpType.is_gt, mybir.AluOpType.subtract)

            # ---- main loop: m-groups x ft sweeps x pairs x token chunks ----
            for g in range(mg):
                if g > 0:
                    for pi, (wa, xa) in enumerate(pairs):
                        xt[(g, pi)] = xpool.tile([128, 2, m_grp], fp8,
                                                 tag="xt", name=f"xt{g}_{pi}")
                        nc.sync.dma_start(
                            xt[(g, pi)][:],
                            xS[xa * 128:(xa + 2) * 128, ts(g, m_grp)]
                            .rearrange("(t p) m -> p t m", p=128))
                for ft in range(nft):
                    ps = [pmain.tile([128, 512], f32, tag="ps",
                                     name=f"ps{b}") for b in range(nbank)]
                    for pi, (wa, xa) in enumerate(pairs):
                        for b in range(nbank):
                            for h in range(2):
                                nc.tensor.matmul(
                                    ps[b][:, ts(h, TCH)],
                                    wqs[:, wa:wa + 2, ts(ft, 128)],
                                    xt[(g, pi)][:, :, ts(2 * b + h, TCH)],
                                    start=(pi == 0 and h == 0),
                                    stop=(pi == npair - 1 and h == 1),
                                    perf_mode=mybir.MatmulPerfMode.DoubleRow)
                    for b in range(nbank):
                        osb = opool.tile([128, 512], bf16, tag="osb")
                        nc.vector.tensor_copy(osb[:], ps[b][:])
                        nc.gpsimd.dma_start(
                            outT[ts(ft, 128), g * m_grp + b * 512:
                                 g * m_grp + (b + 1) * 512], osb[:])

    nc.compile()
    meta = dict(m_grp=m_grp, mg=mg, n_core=n_core, kt=kt,
                corr_start=corr_start, ncores=ncores)
    return nc, meta


def _get_compiled():
    global _COMPILED
    if _COMPILED is None:
        _COMPILED = build_module()
    return _COMPILED


def make_in_maps(x, W):
    """Host-side shard prep. x [B,S,D_IN] f32, W [D_OUT,D_IN] f32."""
    x2 = np.asarray(x, dtype=np.float32).reshape(M_TOTAL, D_IN)
    WTf = np.ascontiguousarray(np.asarray(W, dtype=np.float32).T)  # [k, n]

    xs_groups = []
    for t in range(T_SHARD):
        xT = np.ascontiguousarray(x2[t * M_CORE:(t + 1) * M_CORE, :].T)
        hi = xT.astype(ml_dtypes.float8_e4m3)
        lo = (xT - hi.astype(np.float32)).astype(ml_dtypes.float8_e4m3)
        xs = np.concatenate(
            [hi, lo[CORR_START * 128:, :]], axis=0)  # [NSLAB*128, M_CORE]
        xs_groups.append(np.ascontiguousarray(xs))

    in_maps = []
    for c in range(NCORES):
        t, f = c // F_SHARD, c % F_SHARD
        WTc = np.ascontiguousarray(WTf[:, f * N_CORE:(f + 1) * N_CORE])
        Wgc = np.ascontiguousarray(WTf[:, c * NGAMMA:(c + 1) * NGAMMA])
        in_maps.append({"xS": xs_groups[t], "WT": WTc, "Wg": Wgc})
    return in_maps


def kernel(input, W):
    """Full inputs in, full output out. Shards internally across 8 cores."""
    global LAST_RESULTS
    from concourse import bass_utils

    nc, meta = _get_compiled()
    in_maps = make_in_maps(input, W)
    res = bass_utils.run_bass_kernel_spmd(
        nc, in_maps, core_ids=list(range(NCORES)))
    LAST_RESULTS = res
    out = np.empty((M_TOTAL, D_OUT), dtype=np.float32)
    for c in range(NCORES):
        t, f = c // F_SHARD, c % F_SHARD
        out[t * M_CORE:(t + 1) * M_CORE,
            f * N_CORE:(f + 1) * N_CORE] = \
            res.results[c]["outT"].astype(np.float32).T
    return out.reshape(B, S, D_OUT)
